# revision 2
# baseline (speedup 1.0000x reference)
"""GATv2 layer on 8 Trainium2 NeuronCores (Bass/Tile).

Strategy: sort edges by dst node on host; core k owns dst nodes
[2500k, 2500(k+1)) so segment softmax + aggregation are core-local (no
collectives). Edges are bucketed into 20 blocks of 128 dst nodes per core and
padded per block to a uniform tile count so one SPMD program serves all cores.

Per core on device:
  phase 1: node projections  hsv = [x@W1.T | 0.2*att-dot | x@W.T] (bf16, DRAM)
           hd  = [x@W2.T | 0.2*att-dot] for local dst nodes (bf16, DRAM)
  phase 2: per 128-edge tile: indirect-gather hsv[src], hd[dst];
           z = hs + hd (also alpha_s + alpha_d for the linear logit term);
           zT via PE transpose; r = Relu(zT) (ACT);
           logits = 0.8*att.T @ r  (+ linear term)  [PE, att folded into rhs]
           exp (ACT); weighted = exp * vals (DVE);
           segment-sum via one-hot matmul accumulated in PSUM per block;
           normalize by denominator + bias, DMA out.
"""
import os
import sys

sys.path.insert(0, '/opt/trn_rl_repo')

import numpy as np
import ml_dtypes

N = 20000
IN_F = 128
HEADS = 8
OUT_F = 32
HF = 256          # HEADS * OUT_F
NEG = 0.2
CORES = 8
NPC = 2500        # dst nodes per core
BLOCKS = 20       # 128-node blocks per core (2560 >= 2500)
NLOC = BLOCKS * 128
NT_GLOB = 157     # ceil(20000/128)
NPADG = NT_GLOB * 128

bf16 = ml_dtypes.bfloat16

_CACHE = {}
LAST_EXEC_NS = None


def _build(T_blk):
    import concourse.bass as bass
    from concourse import mybir, bacc
    from concourse.tile import TileContext

    f32 = mybir.dt.float32
    b16 = mybir.dt.bfloat16
    i32 = mybir.dt.int32
    AF = mybir.ActivationFunctionType
    ALU = mybir.AluOpType

    n_tiles = BLOCKS * T_blk
    n_super = n_tiles // 8

    nc = bacc.Bacc("TRN2", target_bir_lowering=False, debug=False,
                   num_devices=CORES)
    xT = nc.dram_tensor("xt", [128, NPADG], f32, kind="ExternalInput")
    xTl = nc.dram_tensor("xtl", [128, NLOC], f32, kind="ExternalInput")
    wcat = nc.dram_tensor("wcat", [128, 520], f32, kind="ExternalInput")
    w2cat = nc.dram_tensor("w2cat", [128, 264], f32, kind="ExternalInput")
    attb = nc.dram_tensor("attblk", [128, 16], b16, kind="ExternalInput")
    ident = nc.dram_tensor("ident", [128, 128], b16, kind="ExternalInput")
    iota = nc.dram_tensor("iota", [128, 1024], f32, kind="ExternalInput")
    biasr = nc.dram_tensor("biasr", [128, 256], f32, kind="ExternalInput")
    srcc = nc.dram_tensor("srcc", [128, n_tiles], i32, kind="ExternalInput")
    ldstc = nc.dram_tensor("ldstc", [128, n_tiles], i32, kind="ExternalInput")
    qdstc = nc.dram_tensor("qdstc", [128, n_tiles], f32, kind="ExternalInput")
    outt = nc.dram_tensor("out", [NLOC, 256], f32, kind="ExternalOutput")

    hsv_d = nc.dram_tensor("hsvd", [NPADG, 520], b16, kind="Internal")
    hd_d = nc.dram_tensor("hdd", [NLOC, 264], b16, kind="Internal")

    with TileContext(nc) as tc:
        with tc.tile_pool(name="const", bufs=1) as cp:
            wcat_sb = cp.tile([128, 520], f32)
            nc.sync.dma_start(wcat_sb[:], wcat[:])
            w2_sb = cp.tile([128, 264], f32)
            nc.sync.dma_start(w2_sb[:], w2cat[:])
            attb_sb = cp.tile([128, 16], b16)
            nc.sync.dma_start(attb_sb[:], attb[:])
            id_sb = cp.tile([128, 128], b16)
            nc.sync.dma_start(id_sb[:], ident[:])
            iota_sb = cp.tile([128, 1024], f32)
            nc.sync.dma_start(iota_sb[:], iota[:])
            bias_sb = cp.tile([128, 256], f32)
            nc.sync.dma_start(bias_sb[:], biasr[:])
            src_sb = cp.tile([128, n_tiles], i32)
            nc.sync.dma_start(src_sb[:], srcc[:])
            ld_sb = cp.tile([128, n_tiles], i32)
            nc.sync.dma_start(ld_sb[:], ldstc[:])
            qd_sb = cp.tile([128, n_tiles], f32)
            nc.sync.dma_start(qd_sb[:], qdstc[:])

            # ---------------- phase 1: projections ----------------
            with tc.tile_pool(name="proj", bufs=4) as pp, \
                 tc.tile_pool(name="pps", bufs=4, space="PSUM") as pps:
                for j in range(NT_GLOB):
                    xt_t = pp.tile([128, 128], f32)
                    nc.sync.dma_start(xt_t[:], xT[:, j * 128:(j + 1) * 128])
                    psA = pps.tile([128, 264], f32)
                    nc.tensor.matmul(psA[:], lhsT=xt_t[:],
                                     rhs=wcat_sb[:, 0:264],
                                     start=True, stop=True)
                    psB = pps.tile([128, 256], f32)
                    nc.tensor.matmul(psB[:], lhsT=xt_t[:],
                                     rhs=wcat_sb[:, 264:520],
                                     start=True, stop=True)
                    hv = pp.tile([128, 520], b16)
                    nc.scalar.copy(hv[:, 0:264], psA[:])
                    nc.vector.tensor_copy(hv[:, 264:520], psB[:])
                    nc.sync.dma_start(hsv_d[j * 128:(j + 1) * 128, :], hv[:])
                for j in range(BLOCKS):
                    xt_t = pp.tile([128, 128], f32)
                    nc.sync.dma_start(xt_t[:], xTl[:, j * 128:(j + 1) * 128])
                    psA = pps.tile([128, 264], f32)
                    nc.tensor.matmul(psA[:], lhsT=xt_t[:], rhs=w2_sb[:],
                                     start=True, stop=True)
                    hv2 = pp.tile([128, 264], b16)
                    nc.vector.tensor_copy(hv2[:], psA[:])
                    nc.sync.dma_start(hd_d[j * 128:(j + 1) * 128, :], hv2[:])

            # ---------------- phase 2: edges ----------------
            with tc.tile_pool(name="edge", bufs=3) as ep, \
                 tc.tile_pool(name="pair", bufs=3) as rp, \
                 tc.tile_pool(name="zps", bufs=3, space="PSUM") as zps, \
                 tc.tile_pool(name="lps", bufs=3, space="PSUM") as lps, \
                 tc.tile_pool(name="aps", bufs=2, space="PSUM") as aps, \
                 tc.tile_pool(name="np_", bufs=2) as npl:
                aggp = None
                for s in range(n_super):
                    hsv_g = ep.tile([128, 8 * 520], b16, tag="hsvg")
                    hd_g = ep.tile([128, 8 * 264], b16, tag="hdg")
                    for t in range(8):
                        g = s * 8 + t
                        nc.gpsimd.indirect_dma_start(
                            out=hsv_g[:, t * 520:(t + 1) * 520],
                            out_offset=None, in_=hsv_d[:],
                            in_offset=bass.IndirectOffsetOnAxis(
                                ap=src_sb[:, g:g + 1], axis=0))
                        nc.gpsimd.indirect_dma_start(
                            out=hd_g[:, t * 264:(t + 1) * 264],
                            out_offset=None, in_=hd_d[:],
                            in_offset=bass.IndirectOffsetOnAxis(
                                ap=ld_sb[:, g:g + 1], axis=0))
                    # z + alpha sums: [128, 8, 264]
                    zx = ep.tile([128, 8 * 264], b16, tag="zx")
                    nc.vector.tensor_add(
                        zx[:].rearrange("p (t c) -> p t c", t=8),
                        hsv_g[:].rearrange("p (t c) -> p t c", t=8)[:, :, 0:264],
                        hd_g[:].rearrange("p (t c) -> p t c", t=8))
                    # one-hot [128, 8, 128]
                    oh = ep.tile([128, 8 * 128], b16, tag="oh")
                    nc.vector.tensor_tensor(
                        out=oh[:].rearrange("p (t c) -> p t c", t=8),
                        in0=iota_sb[:].rearrange("p (t c) -> p t c", t=8),
                        in1=qd_sb[:, s * 8:(s + 1) * 8].unsqueeze(2)
                            .broadcast_to([128, 8, 128]),
                        op=ALU.is_equal)
                    wv = ep.tile([128, 8 * 264], b16, tag="wv")
                    for q in range(4):   # pairs within super
                        t0 = 2 * q
                        zTp = zps.tile([128, 512], f32)
                        for sl in range(2):
                            t = t0 + sl
                            base = t * 264
                            nc.tensor.matmul(
                                zTp[:, sl * 256:sl * 256 + 128],
                                lhsT=zx[:, base:base + 128],
                                rhs=id_sb[:], start=True, stop=True)
                            nc.tensor.matmul(
                                zTp[:, sl * 256 + 128:sl * 256 + 256],
                                lhsT=zx[:, base + 128:base + 256],
                                rhs=id_sb[:], start=True, stop=True)
                        rT = rp.tile([128, 512], b16, tag="rT")
                        nc.scalar.activation(rT[:], zTp[:], AF.Relu)
                        lgt = lps.tile([128, 16], f32)
                        for sl in range(2):
                            t = t0 + sl
                            nc.tensor.matmul(
                                lgt[:, sl * 8:(sl + 1) * 8],
                                lhsT=rT[:, sl * 256:sl * 256 + 128],
                                rhs=attb_sb[:, 0:8], start=True, stop=False)
                            nc.tensor.matmul(
                                lgt[:, sl * 8:(sl + 1) * 8],
                                lhsT=rT[:, sl * 256 + 128:sl * 256 + 256],
                                rhs=attb_sb[:, 8:16], start=False, stop=False)
                            nc.tensor.matmul(
                                lgt[:, sl * 8:(sl + 1) * 8],
                                lhsT=id_sb[:],
                                rhs=zx[:, t * 264 + 256:(t + 1) * 264],
                                start=False, stop=True)
                        # exp -> wv[:, {t0,t0+1}, 256:264]
                        nc.scalar.activation(
                            wv[:].rearrange("p (t c) -> p t c", t=8)
                                [:, t0:t0 + 2, 256:264],
                            lgt[:].rearrange("p (a b) -> p a b", a=2),
                            AF.Exp)
                    # weighted = vals * exp  [128, 8, 8, 32]
                    nc.vector.tensor_tensor(
                        out=wv[:].rearrange("p (t c) -> p t c", t=8)
                            [:, :, 0:256].rearrange(
                                "p t (h f) -> p t h f", h=8),
                        in0=hsv_g[:].rearrange("p (t c) -> p t c", t=8)
                            [:, :, 264:520].rearrange(
                                "p t (h f) -> p t h f", h=8),
                        in1=wv[:].rearrange("p (t c) -> p t c", t=8)
                            [:, :, 256:264].unsqueeze(3)
                            .broadcast_to([128, 8, 8, 32]),
                        op=ALU.mult)
                    for t in range(8):
                        g = s * 8 + t
                        if g % T_blk == 0:
                            aggp = aps.tile([128, 264], f32, tag="agg")
                        nc.tensor.matmul(
                            aggp[:],
                            lhsT=oh[:, t * 128:(t + 1) * 128],
                            rhs=wv[:, t * 264:(t + 1) * 264],
                            start=(g % T_blk == 0),
                            stop=(g % T_blk == T_blk - 1))
                        if g % T_blk == T_blk - 1:
                            b = g // T_blk
                            dn = npl.tile([128, 8], f32, tag="dn")
                            nc.vector.tensor_scalar(
                                out=dn[:], in0=aggp[:, 256:264],
                                scalar1=1e-12, scalar2=None, op0=ALU.max)
                            rec = npl.tile([128, 8], f32, tag="rec")
                            nc.vector.reciprocal(rec[:], dn[:])
                            osb = npl.tile([128, 256], f32, tag="osb")
                            nc.vector.tensor_tensor(
                                out=osb[:].rearrange("p (h f) -> p h f", h=8),
                                in0=aggp[:, 0:256].rearrange(
                                    "p (h f) -> p h f", h=8),
                                in1=rec[:].unsqueeze(2)
                                    .broadcast_to([128, 8, 32]),
                                op=ALU.mult)
                            nc.vector.tensor_add(osb[:], osb[:], bias_sb[:])
                            nc.sync.dma_start(
                                outt[b * 128:(b + 1) * 128, :], osb[:])
    nc.compile()
    return nc


def _prep(x, edge_index, W, W1, W2, att, bias):
    x = np.asarray(x, np.float32)
    ei = np.asarray(edge_index)
    W = np.asarray(W, np.float32)
    W1 = np.asarray(W1, np.float32)
    W2 = np.asarray(W2, np.float32)
    att = np.asarray(att, np.float32)
    bias = np.asarray(bias, np.float32)

    src = ei[0].astype(np.int64)
    dst = ei[1].astype(np.int64)
    perm = np.argsort(dst, kind='stable')
    src_s = src[perm].astype(np.int32)
    dst_s = dst[perm].astype(np.int32)

    # per (core, block) counts
    blk_of = dst_s // 128          # global 128-blocks: 157 of them; but per
    core_of = dst_s // NPC
    # local block index within core
    lblk = (dst_s - core_of * NPC) // 128
    cnt = np.zeros((CORES, BLOCKS), np.int64)
    np.add.at(cnt, (core_of, lblk), 1)
    T_blk = int(np.ceil(cnt.max() / 128))
    if T_blk % 2:
        T_blk += 1
    n_tiles = BLOCKS * T_blk

    # padded per-core edge arrays
    srcc = np.zeros((CORES, n_tiles * 128), np.int32)
    ldst = np.zeros((CORES, n_tiles * 128), np.int32)
    qdst = np.full((CORES, n_tiles * 128), -1.0, np.float32)
    order = np.lexsort((np.arange(len(dst_s)), lblk, core_of))  # stable
    # edges already sorted by dst -> core_of/lblk sorted; just use ranges
    for k in range(CORES):
        for b in range(BLOCKS):
            c = cnt[k, b]
            if c == 0:
                continue
            # contiguous range in sorted arrays
            # start index: edges with (core<k) + (core==k, blk<b)
            lo = np.searchsorted(dst_s, k * NPC + b * 128)
            hi = lo + c
            base = b * T_blk * 128
            srcc[k, base:base + c] = src_s[lo:hi]
            ld = dst_s[lo:hi] - k * NPC
            ldst[k, base:base + c] = ld
            qdst[k, base:base + c] = (ld - b * 128).astype(np.float32)

    # constants
    was02 = NEG * np.einsum('ihf,hf->ih',
                            W1.T.reshape(IN_F, HEADS, OUT_F), att[0])
    wad02 = NEG * np.einsum('ihf,hf->ih',
                            W2.T.reshape(IN_F, HEADS, OUT_F), att[0])
    wcat = np.concatenate([W1.T, was02, W.T], axis=1).astype(np.float32)
    w2cat = np.concatenate([W2.T, wad02], axis=1).astype(np.float32)

    attb = np.zeros((128, 16), np.float32)
    for p in range(128):
        attb[p, p // 32] = (1.0 - NEG) * att[0, p // 32, p % 32]
        attb[p, 8 + 4 + p // 32] = (1.0 - NEG) * att[0, 4 + p // 32, p % 32]
    attb = attb.astype(bf16)

    x_pad = np.zeros((NPADG, IN_F), np.float32)
    x_pad[:N] = x
    xT = np.ascontiguousarray(x_pad.T)
    iota = np.tile(np.arange(128, dtype=np.float32), (128, 8))
    biasr = np.tile(bias[None, :], (128, 1)).astype(np.float32)
    ident = np.eye(128, dtype=np.float32).astype(bf16)

    in_maps = []
    for k in range(CORES):
        xl = np.ascontiguousarray(
            x_pad[k * NPC:k * NPC + NLOC].T).astype(np.float32)
        in_maps.append({
            "xt": xT, "xtl": xl, "wcat": wcat, "w2cat": w2cat,
            "attblk": attb, "ident": ident, "iota": iota, "biasr": biasr,
            "srcc": np.ascontiguousarray(
                srcc[k].reshape(n_tiles, 128).T),
            "ldstc": np.ascontiguousarray(
                ldst[k].reshape(n_tiles, 128).T),
            "qdstc": np.ascontiguousarray(
                qdst[k].reshape(n_tiles, 128).T),
        })
    return T_blk, in_maps


def kernel(x, edge_index, W, W1, W2, att, bias):
    global LAST_EXEC_NS
    from concourse import bass_utils

    T_blk, in_maps = _prep(x, edge_index, W, W1, W2, att, bias)
    if T_blk not in _CACHE:
        _CACHE[T_blk] = _build(T_blk)
    nc = _CACHE[T_blk]

    trace = bool(int(os.environ.get("GAT_TRACE", "0")))
    res = bass_utils.run_bass_kernel_spmd(
        nc, in_maps, core_ids=list(range(CORES)), trace=trace)
    LAST_EXEC_NS = res.exec_time_ns

    out = np.empty((N, HF), np.float32)
    for k in range(CORES):
        out[k * NPC:(k + 1) * NPC] = res.results[k]["out"][:NPC]
    return out


# revision 4
# speedup vs baseline: 156.8339x; 156.8339x over previous
"""GATv2 layer on 8 Trainium2 NeuronCores (Bass/Tile).

Strategy: sort edges by dst node on host; core k owns dst nodes
[2500k, 2500(k+1)) so segment softmax + aggregation are core-local (no
collectives). Edges are bucketed into 20 blocks of 128 dst nodes per core and
padded per block to a uniform tile count so one SPMD program serves all cores.

Per core on device:
  phase 1: node projections  hsv = [x@W1.T | 0.2*att-dot | x@W.T] (bf16, DRAM)
           hd  = [x@W2.T | 0.2*att-dot] for local dst nodes (bf16, DRAM)
  phase 2: per 128-edge tile: indirect-gather hsv[src], hd[dst];
           z = hs + hd (also alpha_s + alpha_d for the linear logit term);
           zT via PE transpose; r = Relu(zT) (ACT);
           logits = 0.8*att.T @ r  (+ linear term)  [PE, att folded into rhs]
           exp (ACT); weighted = exp * vals (DVE);
           segment-sum via one-hot matmul accumulated in PSUM per block;
           normalize by denominator + bias, DMA out.
"""
import os
import sys

sys.path.insert(0, '/opt/trn_rl_repo')

import numpy as np
import ml_dtypes

N = 20000
IN_F = 128
HEADS = 8
OUT_F = 32
HF = 256          # HEADS * OUT_F
NEG = 0.2
CORES = 8
NPC = 2500        # dst nodes per core
BLOCKS = 20       # 128-node blocks per core (2560 >= 2500)
NLOC = BLOCKS * 128
NT_GLOB = 157     # ceil(20000/128)
NPADG = NT_GLOB * 128

bf16 = ml_dtypes.bfloat16

_CACHE = {}
LAST_EXEC_NS = None


def _build(T_blk):
    import concourse.bass as bass
    from concourse import mybir, bacc
    from concourse.tile import TileContext

    f32 = mybir.dt.float32
    b16 = mybir.dt.bfloat16
    i32 = mybir.dt.int32
    AF = mybir.ActivationFunctionType
    ALU = mybir.AluOpType

    n_tiles = BLOCKS * T_blk
    n_super = n_tiles // 8

    nc = bacc.Bacc("TRN2", target_bir_lowering=False, debug=False,
                   num_devices=CORES)
    xT = nc.dram_tensor("xt", [128, NPADG], f32, kind="ExternalInput")
    xTl = nc.dram_tensor("xtl", [128, NLOC], f32, kind="ExternalInput")
    wcat = nc.dram_tensor("wcat", [128, 520], f32, kind="ExternalInput")
    w2cat = nc.dram_tensor("w2cat", [128, 264], f32, kind="ExternalInput")
    attb = nc.dram_tensor("attblk", [128, 16], b16, kind="ExternalInput")
    ident = nc.dram_tensor("ident", [128, 128], b16, kind="ExternalInput")
    iota = nc.dram_tensor("iota", [128, 1024], f32, kind="ExternalInput")
    biasr = nc.dram_tensor("biasr", [128, 256], f32, kind="ExternalInput")
    srcc = nc.dram_tensor("srcc", [128, n_tiles], i32, kind="ExternalInput")
    qdstc = nc.dram_tensor("qdstc", [128, n_tiles], f32, kind="ExternalInput")
    qdT = nc.dram_tensor("qdt", [128, n_tiles * 128], f32, kind="ExternalInput")
    iotaP = nc.dram_tensor("iotap", [128, 1024], f32, kind="ExternalInput")
    outt = nc.dram_tensor("out", [NLOC, 256], f32, kind="ExternalOutput")

    hsv_d = nc.dram_tensor("hsvd", [NPADG, 520], b16, kind="Internal")

    with TileContext(nc) as tc:
        with tc.tile_pool(name="const", bufs=1) as cp:
            wcat_sb = cp.tile([128, 520], f32)
            nc.sync.dma_start(wcat_sb[:], wcat[:])
            w2_sb = cp.tile([128, 264], f32)
            nc.sync.dma_start(w2_sb[:], w2cat[:])
            attb_sb = cp.tile([128, 16], b16)
            nc.sync.dma_start(attb_sb[:], attb[:])
            id_sb = cp.tile([128, 128], b16)
            nc.sync.dma_start(id_sb[:], ident[:])
            iota_sb = cp.tile([128, 1024], f32)
            nc.sync.dma_start(iota_sb[:], iota[:])
            bias_sb = cp.tile([128, 256], f32)
            nc.sync.dma_start(bias_sb[:], biasr[:])
            src_sb = cp.tile([128, n_tiles], i32)
            nc.sync.dma_start(src_sb[:], srcc[:])
            iop_sb = cp.tile([128, 1024], f32)
            nc.sync.dma_start(iop_sb[:], iotaP[:])
            qd_sb = cp.tile([128, n_tiles], f32)
            nc.sync.dma_start(qd_sb[:], qdstc[:])

            # ---------------- phase 1: projections ----------------
            with tc.tile_pool(name="proj", bufs=4) as pp, \
                 tc.tile_pool(name="pps", bufs=4, space="PSUM") as pps:
                for j in range(NT_GLOB):
                    xt_t = pp.tile([128, 128], f32)
                    nc.sync.dma_start(xt_t[:], xT[:, j * 128:(j + 1) * 128])
                    psA = pps.tile([128, 264], f32)
                    nc.tensor.matmul(psA[:], lhsT=xt_t[:],
                                     rhs=wcat_sb[:, 0:264],
                                     start=True, stop=True)
                    psB = pps.tile([128, 256], f32)
                    nc.tensor.matmul(psB[:], lhsT=xt_t[:],
                                     rhs=wcat_sb[:, 264:520],
                                     start=True, stop=True)
                    hv = pp.tile([128, 520], b16)
                    nc.scalar.copy(hv[:, 0:264], psA[:])
                    nc.vector.tensor_copy(hv[:, 264:520], psB[:])
                    nc.sync.dma_start(hsv_d[j * 128:(j + 1) * 128, :], hv[:])
                hd_tiles = []
                for j in range(BLOCKS):
                    xt_t = pp.tile([128, 128], f32)
                    nc.sync.dma_start(xt_t[:], xTl[:, j * 128:(j + 1) * 128])
                    psA = pps.tile([128, 264], f32)
                    nc.tensor.matmul(psA[:], lhsT=xt_t[:], rhs=w2_sb[:],
                                     start=True, stop=True)
                    hv2 = cp.tile([128, 264], b16, tag=f"hd{j}")
                    nc.vector.tensor_copy(hv2[:], psA[:])
                    hd_tiles.append(hv2)

            # ---------------- phase 2: edges ----------------
            with tc.tile_pool(name="edge", bufs=3) as ep, \
                 tc.tile_pool(name="pair", bufs=3) as rp, \
                 tc.tile_pool(name="zps", bufs=3, space="PSUM") as zps, \
                 tc.tile_pool(name="lps", bufs=3, space="PSUM") as lps, \
                 tc.tile_pool(name="aps", bufs=2, space="PSUM") as aps, \
                 tc.tile_pool(name="np_", bufs=2) as npl:
                aggp = None
                for s in range(n_super):
                    hsv_g = ep.tile([128, 8 * 520], b16, tag="hsvg")
                    for t in range(8):
                        g = s * 8 + t
                        nc.gpsimd.indirect_dma_start(
                            out=hsv_g[:, t * 520:(t + 1) * 520],
                            out_offset=None, in_=hsv_d[:],
                            in_offset=bass.IndirectOffsetOnAxis(
                                ap=src_sb[:, g:g + 1], axis=0))
                    qdt_sb = ep.tile([128, 1024], f32, tag="qdt")
                    nc.sync.dma_start(
                        qdt_sb[:], qdT[:, s * 1024:(s + 1) * 1024])
                    ohT = ep.tile([128, 8 * 128], b16, tag="ohT")
                    nc.vector.tensor_tensor(
                        out=ohT[:], in0=qdt_sb[:], in1=iop_sb[:],
                        op=ALU.is_equal)
                    # one-hot [128, 8, 128]
                    oh = ep.tile([128, 8 * 128], b16, tag="oh")
                    nc.vector.tensor_tensor(
                        out=oh[:].rearrange("p (t c) -> p t c", t=8),
                        in0=iota_sb[:].rearrange("p (t c) -> p t c", t=8),
                        in1=qd_sb[:, s * 8:(s + 1) * 8].unsqueeze(2)
                            .broadcast_to([128, 8, 128]),
                        op=ALU.is_equal)
                    wv = ep.tile([128, 8 * 264], b16, tag="wv")
                    for q in range(4):   # pairs within super
                        t0 = 2 * q
                        zTp = zps.tile([128, 512], f32)
                        blk = (s * 8 + t0) // T_blk
                        hdt = hd_tiles[blk]
                        for sl in range(2):
                            t = t0 + sl
                            base = t * 520
                            for hf in range(2):
                                dst_sl = zTp[:, sl * 256 + hf * 128:
                                             sl * 256 + hf * 128 + 128]
                                nc.tensor.matmul(
                                    dst_sl,
                                    lhsT=hsv_g[:, base + hf * 128:
                                               base + hf * 128 + 128],
                                    rhs=id_sb[:], start=True, stop=False)
                                nc.tensor.matmul(
                                    dst_sl,
                                    lhsT=hdt[:, hf * 128:hf * 128 + 128],
                                    rhs=ohT[:, t * 128:(t + 1) * 128],
                                    start=False, stop=True)
                        rT = rp.tile([128, 512], b16, tag="rT")
                        nc.scalar.activation(rT[:], zTp[:], AF.Relu)
                        lgt = lps.tile([128, 16], f32)
                        for sl in range(2):
                            t = t0 + sl
                            nc.tensor.matmul(
                                lgt[:, sl * 8:(sl + 1) * 8],
                                lhsT=rT[:, sl * 256:sl * 256 + 128],
                                rhs=attb_sb[:, 0:8], start=True, stop=False)
                            nc.tensor.matmul(
                                lgt[:, sl * 8:(sl + 1) * 8],
                                lhsT=rT[:, sl * 256 + 128:sl * 256 + 256],
                                rhs=attb_sb[:, 8:16], start=False, stop=False)
                            nc.tensor.matmul(
                                lgt[:, sl * 8:(sl + 1) * 8],
                                lhsT=id_sb[:],
                                rhs=hsv_g[:, t * 520 + 256:t * 520 + 264],
                                start=False, stop=False)
                            nc.tensor.matmul(
                                lgt[:, sl * 8:(sl + 1) * 8],
                                lhsT=ohT[:, t * 128:(t + 1) * 128],
                                rhs=hdt[:, 256:264],
                                start=False, stop=True)
                        # exp -> wv[:, {t0,t0+1}, 256:264]
                        nc.scalar.activation(
                            wv[:].rearrange("p (t c) -> p t c", t=8)
                                [:, t0:t0 + 2, 256:264],
                            lgt[:].rearrange("p (a b) -> p a b", a=2),
                            AF.Exp)
                    # weighted = vals * exp  [128, 8, 8, 32]
                    nc.vector.tensor_tensor(
                        out=wv[:].rearrange("p (t c) -> p t c", t=8)
                            [:, :, 0:256].rearrange(
                                "p t (h f) -> p t h f", h=8),
                        in0=hsv_g[:].rearrange("p (t c) -> p t c", t=8)
                            [:, :, 264:520].rearrange(
                                "p t (h f) -> p t h f", h=8),
                        in1=wv[:].rearrange("p (t c) -> p t c", t=8)
                            [:, :, 256:264].unsqueeze(3)
                            .broadcast_to([128, 8, 8, 32]),
                        op=ALU.mult)
                    for t in range(8):
                        g = s * 8 + t
                        if g % T_blk == 0:
                            aggp = aps.tile([128, 264], f32, tag="agg")
                        nc.tensor.matmul(
                            aggp[:],
                            lhsT=oh[:, t * 128:(t + 1) * 128],
                            rhs=wv[:, t * 264:(t + 1) * 264],
                            start=(g % T_blk == 0),
                            stop=(g % T_blk == T_blk - 1))
                        if g % T_blk == T_blk - 1:
                            b = g // T_blk
                            dn = npl.tile([128, 8], f32, tag="dn")
                            nc.vector.tensor_scalar(
                                out=dn[:], in0=aggp[:, 256:264],
                                scalar1=1e-12, scalar2=None, op0=ALU.max)
                            rec = npl.tile([128, 8], f32, tag="rec")
                            nc.vector.reciprocal(rec[:], dn[:])
                            osb = npl.tile([128, 256], f32, tag="osb")
                            nc.vector.tensor_tensor(
                                out=osb[:].rearrange("p (h f) -> p h f", h=8),
                                in0=aggp[:, 0:256].rearrange(
                                    "p (h f) -> p h f", h=8),
                                in1=rec[:].unsqueeze(2)
                                    .broadcast_to([128, 8, 32]),
                                op=ALU.mult)
                            nc.vector.tensor_add(osb[:], osb[:], bias_sb[:])
                            nc.sync.dma_start(
                                outt[b * 128:(b + 1) * 128, :], osb[:])
    nc.compile()
    return nc


def _prep(x, edge_index, W, W1, W2, att, bias):
    x = np.asarray(x, np.float32)
    ei = np.asarray(edge_index)
    W = np.asarray(W, np.float32)
    W1 = np.asarray(W1, np.float32)
    W2 = np.asarray(W2, np.float32)
    att = np.asarray(att, np.float32)
    bias = np.asarray(bias, np.float32)

    src = ei[0].astype(np.int64)
    dst = ei[1].astype(np.int64)
    perm = np.argsort(dst, kind='stable')
    src_s = src[perm].astype(np.int32)
    dst_s = dst[perm].astype(np.int32)

    # per (core, block) counts
    blk_of = dst_s // 128          # global 128-blocks: 157 of them; but per
    core_of = dst_s // NPC
    # local block index within core
    lblk = (dst_s - core_of * NPC) // 128
    cnt = np.zeros((CORES, BLOCKS), np.int64)
    np.add.at(cnt, (core_of, lblk), 1)
    T_blk = int(np.ceil(cnt.max() / 128))
    if T_blk % 2:
        T_blk += 1
    n_tiles = BLOCKS * T_blk

    # padded per-core edge arrays
    srcc = np.zeros((CORES, n_tiles * 128), np.int32)
    ldst = np.zeros((CORES, n_tiles * 128), np.int32)
    qdst = np.full((CORES, n_tiles * 128), -1.0, np.float32)
    order = np.lexsort((np.arange(len(dst_s)), lblk, core_of))  # stable
    # edges already sorted by dst -> core_of/lblk sorted; just use ranges
    for k in range(CORES):
        for b in range(BLOCKS):
            c = cnt[k, b]
            if c == 0:
                continue
            # contiguous range in sorted arrays
            # start index: edges with (core<k) + (core==k, blk<b)
            lo = np.searchsorted(dst_s, k * NPC + b * 128)
            hi = lo + c
            base = b * T_blk * 128
            srcc[k, base:base + c] = src_s[lo:hi]
            ld = dst_s[lo:hi] - k * NPC
            ldst[k, base:base + c] = ld
            qdst[k, base:base + c] = (ld - b * 128).astype(np.float32)

    # constants
    was02 = NEG * np.einsum('ihf,hf->ih',
                            W1.T.reshape(IN_F, HEADS, OUT_F), att[0])
    wad02 = NEG * np.einsum('ihf,hf->ih',
                            W2.T.reshape(IN_F, HEADS, OUT_F), att[0])
    wcat = np.concatenate([W1.T, was02, W.T], axis=1).astype(np.float32)
    w2cat = np.concatenate([W2.T, wad02], axis=1).astype(np.float32)

    attb = np.zeros((128, 16), np.float32)
    for p in range(128):
        attb[p, p // 32] = (1.0 - NEG) * att[0, p // 32, p % 32]
        attb[p, 8 + 4 + p // 32] = (1.0 - NEG) * att[0, 4 + p // 32, p % 32]
    attb = attb.astype(bf16)

    x_pad = np.zeros((NPADG, IN_F), np.float32)
    x_pad[:N] = x
    xT = np.ascontiguousarray(x_pad.T)
    iota = np.tile(np.arange(128, dtype=np.float32), (128, 8))
    iotap = np.ascontiguousarray(
        np.tile(np.arange(128, dtype=np.float32)[:, None], (1, 1024)))
    biasr = np.tile(bias[None, :], (128, 1)).astype(np.float32)
    ident = np.eye(128, dtype=np.float32).astype(bf16)

    in_maps = []
    for k in range(CORES):
        xl = np.ascontiguousarray(
            x_pad[k * NPC:k * NPC + NLOC].T).astype(np.float32)
        in_maps.append({
            "xt": xT, "xtl": xl, "wcat": wcat, "w2cat": w2cat,
            "attblk": attb, "ident": ident, "iota": iota, "biasr": biasr,
            "srcc": np.ascontiguousarray(
                srcc[k].reshape(n_tiles, 128).T),
            "qdstc": np.ascontiguousarray(
                qdst[k].reshape(n_tiles, 128).T),
            "qdt": np.ascontiguousarray(
                np.tile(qdst[k][None, :], (128, 1))),
            "iotap": iotap,
        })
    return T_blk, in_maps


def kernel(x, edge_index, W, W1, W2, att, bias):
    global LAST_EXEC_NS
    from concourse import bass_utils

    T_blk, in_maps = _prep(x, edge_index, W, W1, W2, att, bias)
    if T_blk not in _CACHE:
        _CACHE[T_blk] = _build(T_blk)
    nc = _CACHE[T_blk]

    trace = bool(int(os.environ.get("GAT_TRACE", "0")))
    res = bass_utils.run_bass_kernel_spmd(
        nc, in_maps, core_ids=list(range(CORES)), trace=trace)
    LAST_EXEC_NS = res.exec_time_ns

    out = np.empty((N, HF), np.float32)
    for k in range(CORES):
        out[k * NPC:(k + 1) * NPC] = res.results[k]["out"][:NPC]
    return out
